# revision 1
# baseline (speedup 1.0000x reference)
"""Trainium2 Bass kernel for nn_CrossAttnTimeQueryHead.

Strategy: data-parallel over B (128 -> 16 per core x 8 cores), all weights
replicated.  Host side does pure relayout only (shard slicing, transposes,
broadcast of tiny vectors); all arithmetic runs on-device in bf16 matmuls
with fp32 PSUM accumulation.

Per-core dataflow (B_LOC=16 batches, processed as 8 pairs in 2 groups):
  hT[d,t]   = win^T x^T + pos^T + bin      (x^T fed pre-transposed from host)
  per layer: KT[e,t], V[t,e] from hT; Q^T from q-state (layer0 hoisted),
  scoresT[k,(h,q)] via tile_position-packed K=32 matmuls, softmax without
  max-subtraction (scores are tiny; kb dropped -- softmax shift-invariant),
  exp on ACT with fused SCALE, softmax sums via ones-matmul columns,
  attn@V packed 2 heads/matmul, normalization via per-partition reciprocal,
  o-proj (+ob+vb@ow folded bias), residual+LN in [q,d] layout
  (rstd = exp(-0.5*ln(var+eps)), same ACT table set as softmax exp),
  FFN in ffn1^T layout with exact Gelu, head projection, fp32 output.
"""

import sys
import os
from contextlib import ExitStack

for _p in ("/opt/trn_rl_repo",):
    if _p not in sys.path and os.path.isdir(_p):
        sys.path.insert(0, _p)

import numpy as np

import concourse.bass as bass
import concourse.mybir as mybir
import concourse.tile as tile
from concourse import bacc
from concourse import bass_utils
from concourse.masks import make_identity

F32 = mybir.dt.float32
BF16 = mybir.dt.bfloat16
AF = mybir.ActivationFunctionType

N_CORES = 8
B = 128
B_LOC = B // N_CORES          # 16
T = 1000
D_IN = 512
D = 256
H = 8
HEAD = 32
L = 2
D_FF = 1024
D_OUT = 512
TQ = 64
SCALE = HEAD ** -0.5
EPS = 1e-5
KC = 8                        # k chunks
KCS = T // KC                 # 125
TH = T // 2                   # 500 (t halves for N<=512 matmuls)
PAIRS = B_LOC // 2            # 8
GROUPS = 4
PAIRS_PER_GROUP = PAIRS // GROUPS  # 4

# rows_sb offsets (all bf16 row vectors on partition 0)
RO_QB = 0          # qb: i*D          (2*256)
RO_F1B = 512       # f1b: RO_F1B + i*D_FF (2*1024)
RO_F2B = 2560      # f2b: + i*D
RO_BOUT = 3072     # bout (512)
RO_OB = 3584       # ob raw: + i*D
RO_OBP = 4096      # ob' = ob + vb@ow (computed on chip): + i*D
ROWS_LEN = 4608
GELU = [AF.Gelu]   # swappable for sim (CoreSim lacks Gelu)


def _emit(ctx, tc, outs, ins):
    nc = tc.nc
    out_d = outs["out"]

    # ---------------- pools ----------------
    consts = ctx.enter_context(tc.tile_pool(name="consts", bufs=1))
    stage_p = ctx.enter_context(tc.tile_pool(name="stage", bufs=2))
    xt_p = ctx.enter_context(tc.tile_pool(name="xt", bufs=3))
    ht_p = ctx.enter_context(tc.tile_pool(name="ht", bufs=5))
    kt_p = ctx.enter_context(tc.tile_pool(name="kt", bufs=4))
    v_p = ctx.enter_context(tc.tile_pool(name="v", bufs=4))
    at_p = ctx.enter_context(tc.tile_pool(name="at", bufs=3))
    ao_p = ctx.enter_context(tc.tile_pool(name="ao", bufs=3))
    aot_p = ctx.enter_context(tc.tile_pool(name="aot", bufs=2))
    qs_p = ctx.enter_context(tc.tile_pool(name="qstate", bufs=6))
    ln_p = ctx.enter_context(tc.tile_pool(name="lnout", bufs=6))
    tmp_p = ctx.enter_context(tc.tile_pool(name="tmp", bufs=4))
    small_p = ctx.enter_context(tc.tile_pool(name="small", bufs=8))
    gel_p = ctx.enter_context(tc.tile_pool(name="gel", bufs=3))
    outp_p = ctx.enter_context(tc.tile_pool(name="outp", bufs=3))

    ps_proj = ctx.enter_context(tc.tile_pool(name="psproj", bufs=2, space="PSUM"))
    ps_sc = ctx.enter_context(tc.tile_pool(name="pssc", bufs=3, space="PSUM"))
    ps_av = ctx.enter_context(tc.tile_pool(name="psav", bufs=1, space="PSUM"))
    ps_qs = ctx.enter_context(tc.tile_pool(name="psqs", bufs=2, space="PSUM"))

    # ---------------- constants / weights ----------------
    ones_row = consts.tile([1, 1024], BF16)
    nc.vector.memset(ones_row[:], 1.0)
    ones_col = consts.tile([128, 1], BF16)
    nc.vector.memset(ones_col[:], 1.0)
    zero_col = consts.tile([128, 1], F32)
    nc.vector.memset(zero_col[:], 0.0)
    eps_col = consts.tile([128, 1], F32)
    nc.vector.memset(eps_col[:], EPS)
    id_sb = consts.tile([128, 128], BF16)
    make_identity(nc, id_sb[:])

    win_sb = consts.tile([128, 4 * D], BF16)
    nc.gpsimd.dma_start(
        out=win_sb[:, :].rearrange("p (c e) -> p c e", c=4),
        in_=ins["win"].rearrange("(c p) e -> p c e", p=128))
    bin_col = consts.tile([128, 2], F32)
    for c in range(2):
        nc.sync.dma_start(out=bin_col[:, c:c + 1], in_=ins["binv"][c * 128:(c + 1) * 128])
    # posTb = pos^T + bin  (bf16)
    posTb = consts.tile([128, 2 * T], BF16)
    for c in range(2):
        stg = stage_p.tile([128, T], F32)
        nc.sync.dma_start(out=stg[:], in_=ins["post"][c * 128:(c + 1) * 128, :])
        nc.scalar.activation(out=posTb[:, c * T:(c + 1) * T], in_=stg[:],
                             func=AF.Identity, bias=bin_col[:, c:c + 1], scale=1.0)
    kw_sb = consts.tile([128, L * 2 * D], BF16)
    vw_sb = consts.tile([128, L * 2 * D], BF16)
    qw_sb = consts.tile([128, L * 2 * D], BF16)
    ow_sb = consts.tile([128, L * 2 * D], BF16)
    for nm, tgt in (("kw", kw_sb), ("vw", vw_sb), ("qw", qw_sb), ("ow", ow_sb)):
        nc.gpsimd.dma_start(
            out=tgt[:, :].rearrange("p (i dc e) -> p i dc e", i=L, dc=2),
            in_=ins[nm].rearrange("i (dc p) e -> p i dc e", p=128))
    f1w_sb = consts.tile([128, L * 2 * D_FF], BF16)
    nc.gpsimd.dma_start(
        out=f1w_sb[:, :].rearrange("p (i dc e) -> p i dc e", i=L, dc=2),
        in_=ins["f1w"].rearrange("i (dc p) e -> p i dc e", p=128))
    f2w_sb = consts.tile([128, L * 8 * D], BF16)
    nc.gpsimd.dma_start(
        out=f2w_sb[:, :].rearrange("p (i fc e) -> p i fc e", i=L, fc=8),
        in_=ins["f2w"].rearrange("i (fc p) e -> p i fc e", p=128))
    wout_sb = consts.tile([128, 2 * D_OUT], BF16)
    nc.gpsimd.dma_start(
        out=wout_sb[:, :].rearrange("p (dc e) -> p dc e", dc=2),
        in_=ins["wout"].rearrange("(dc p) e -> p dc e", p=128))
    tqT_sb = consts.tile([128, 128], BF16)
    for c in range(2):
        nc.gpsimd.dma_start(out=tqT_sb[:, c * TQ:(c + 1) * TQ],
                            in_=ins["tqt"][c * 128:(c + 1) * 128, :])
    tqpair_sb = consts.tile([128, D], F32)
    nc.sync.dma_start(out=tqpair_sb[:], in_=ins["tqpair"][:, :])
    lns_sb = consts.tile([128, L * D], F32)
    lnb_sb = consts.tile([128, L * D], F32)
    for i in range(L):
        nc.sync.dma_start(out=lns_sb[:, i * D:(i + 1) * D], in_=ins["lns"][i, :, :])
        nc.sync.dma_start(out=lnb_sb[:, i * D:(i + 1) * D], in_=ins["lnb"][i, :, :])

    rows_sb = consts.tile([1, ROWS_LEN], BF16)
    for i in range(L):
        nc.gpsimd.dma_start(out=rows_sb[0:1, RO_QB + i * D: RO_QB + (i + 1) * D],
                            in_=ins["qb"][i, :])
        nc.gpsimd.dma_start(out=rows_sb[0:1, RO_F1B + i * D_FF: RO_F1B + (i + 1) * D_FF],
                            in_=ins["f1b"][i, :])
        nc.gpsimd.dma_start(out=rows_sb[0:1, RO_F2B + i * D: RO_F2B + (i + 1) * D],
                            in_=ins["f2b"][i, :])
        nc.gpsimd.dma_start(out=rows_sb[0:1, RO_OB + i * D: RO_OB + (i + 1) * D],
                            in_=ins["ob"][i, :])
    nc.gpsimd.dma_start(out=rows_sb[0:1, RO_BOUT: RO_BOUT + D_OUT], in_=ins["bout"][:])

    vb_col = consts.tile([128, 2 * L], BF16)
    for i in range(L):
        for ec in range(2):
            nc.gpsimd.dma_start(out=vb_col[:, i * 2 + ec: i * 2 + ec + 1],
                                in_=ins["vbv"][i, ec * 128:(ec + 1) * 128])


    # ob' = ob + vb @ ow  per layer -> rows_sb[RO_OBP + i*D]
    for i in range(L):
        pso = ps_qs.tile([1, D], F32, tag="qs")
        for ec in range(2):
            nc.tensor.matmul(pso[0:1, :], lhsT=vb_col[:, i * 2 + ec: i * 2 + ec + 1],
                             rhs=ow_sb[:, i * 2 * D + ec * D: i * 2 * D + (ec + 1) * D],
                             start=(ec == 0), stop=False)
        nc.tensor.matmul(pso[0:1, :], lhsT=ones_row[0:1, 0:1],
                         rhs=rows_sb[0:1, RO_OB + i * D: RO_OB + (i + 1) * D],
                         start=False, stop=True)
        nc.vector.tensor_copy(out=rows_sb[0:1, RO_OBP + i * D: RO_OBP + (i + 1) * D],
                              in_=pso[0:1, :])

    # layer-0 block-diag Q (shared across all batches): qbd0 [128, 2*256]
    qbd0 = consts.tile([128, 512], BF16)
    nc.vector.memset(qbd0[:], 0.0)
    for ec in range(2):
        psq = ps_qs.tile([128, 512], F32, tag="qs")
        nc.tensor.matmul(psq[:, 0:TQ], lhsT=rows_sb[0:1, RO_QB + ec * 128: RO_QB + (ec + 1) * 128],
                         rhs=ones_row[0:1, 0:TQ], start=True, stop=False)
        for dc in range(2):
            nc.tensor.matmul(psq[:, 0:TQ],
                             lhsT=qw_sb[:, dc * D + ec * 128: dc * D + (ec + 1) * 128],
                             rhs=tqT_sb[:, dc * TQ:(dc + 1) * TQ],
                             start=False, stop=(dc == 1))
        for hl in range(4):
            nc.vector.tensor_copy(
                out=qbd0[32 * hl:32 * (hl + 1), ec * 256 + hl * TQ: ec * 256 + (hl + 1) * TQ],
                in_=psq[32 * hl:32 * (hl + 1), 0:TQ])

    # ---------------- helpers ----------------
    def pe_transpose_2(src_bf, dst_bf, dst_col_off, col_w):
        """dst[:, dst_col_off + c*col_w*? ...]: transpose src [P,256] into dst chunks.
        src_bf: [rows, 256] bf16; writes dst[:, c*128*?]: for c in 2:
        transpose src[:, c*128:(c+1)*128] -> [128, rows] -> dst[:, dst_col_off + c*col_w : +rows]
        """
        rows = src_bf.shape[0]
        for c in range(2):
            tp = ps_qs.tile([128, 128], BF16, tag="qs")
            nc.tensor.transpose(tp[:, 0:rows], src_bf[0:rows, c * 128:(c + 1) * 128],
                                id_sb[0:rows, 0:rows])
            nc.vector.tensor_copy(out=dst_bf[:, dst_col_off + c * col_w: dst_col_off + c * col_w + rows],
                                  in_=tp[:, 0:rows])

    # ---------------- main loops ----------------
    ht_tiles = [None] * B_LOC
    qstate = [None] * PAIRS     # fp32 [128,256] per pair
    lnout = [None] * PAIRS
    qtp = [None] * PAIRS        # layer-1 Q^T per pair

    for grp in range(GROUPS):
        g_pairs = [grp * PAIRS_PER_GROUP + k for k in range(PAIRS_PER_GROUP)]
        for i in range(L):
            # ======== attention phase ========
            for p in g_pairs:
                if i == 1:
                    # layer-1 block-diag Q per batch from q_state[p]
                    qcast = tmp_p.tile([128, D], BF16, tag="qcast")
                    nc.vector.tensor_copy(out=qcast[:], in_=qstate[p][:])
                    qsT = tmp_p.tile([128, D], BF16, tag="qsT")
                    pe_transpose_2(qcast, qsT, 0, 128)
                    qbd_a = tmp_p.tile([128, 512], BF16, tag="qbd0")
                    qbd_c = tmp_p.tile([128, 512], BF16, tag="qbd1")
                    qbd_pair = [qbd_a, qbd_c]
                    for bb in range(2):
                        nc.vector.memset(qbd_pair[bb][:], 0.0)
                    for ec in range(2):
                        psq = ps_qs.tile([128, 512], F32, tag="qs")
                        nc.tensor.matmul(
                            psq[:, 0:128],
                            lhsT=rows_sb[0:1, RO_QB + D + ec * 128: RO_QB + D + (ec + 1) * 128],
                            rhs=ones_row[0:1, 0:128], start=True, stop=False)
                        for dc in range(2):
                            nc.tensor.matmul(
                                psq[:, 0:128],
                                lhsT=qw_sb[:, 2 * D + dc * D + ec * 128: 2 * D + dc * D + (ec + 1) * 128],
                                rhs=qsT[:, dc * 128:(dc + 1) * 128],
                                start=False, stop=(dc == 1))
                        for bb in range(2):
                            for hl in range(4):
                                nc.vector.tensor_copy(
                                    out=qbd_pair[bb][32 * hl:32 * (hl + 1), ec * 256 + hl * TQ: ec * 256 + (hl + 1) * TQ],
                                    in_=psq[32 * hl:32 * (hl + 1), bb * TQ:(bb + 1) * TQ])
                    qtp[p] = qbd_pair

                aoT = aot_p.tile([128, 2 * 128], BF16, tag="aoT")
                for bb in range(2):
                    b = 2 * p + bb
                    if i == 0:
                        xt = xt_p.tile([128, 4 * T], BF16, tag="xt")
                        for c in range(4):
                            nc.gpsimd.dma_start(out=xt[:, c * T:(c + 1) * T],
                                                in_=ins["xt"][b, c * 128:(c + 1) * 128, :])
                        ht = ht_p.tile([128, 2 * T], BF16, tag="ht")
                        for dc in range(2):
                            for th in range(2):
                                psp = ps_proj.tile([128, 512], F32, tag="proj")
                                for ic in range(4):
                                    nc.tensor.matmul(
                                        psp[:, 0:TH],
                                        lhsT=win_sb[:, ic * D + dc * 128: ic * D + (dc + 1) * 128],
                                        rhs=xt[:, ic * T + th * TH: ic * T + (th + 1) * TH],
                                        start=(ic == 0), stop=(ic == 3))
                                nc.vector.tensor_add(
                                    out=ht[:, dc * T + th * TH: dc * T + (th + 1) * TH],
                                    in0=psp[:, 0:TH],
                                    in1=posTb[:, dc * T + th * TH: dc * T + (th + 1) * TH])
                        ht_tiles[b] = ht
                    ht = ht_tiles[b]

                    # K^T [e(2x128 part), t]
                    kt = kt_p.tile([128, 2 * T], BF16, tag="kt")
                    for ec in range(2):
                        for th in range(2):
                            psp = ps_proj.tile([128, 512], F32, tag="proj")
                            for dc in range(2):
                                nc.tensor.matmul(
                                    psp[:, 0:TH],
                                    lhsT=kw_sb[:, i * 2 * D + dc * D + ec * 128: i * 2 * D + dc * D + (ec + 1) * 128],
                                    rhs=ht[:, dc * T + th * TH: dc * T + (th + 1) * TH],
                                    start=(dc == 0), stop=(dc == 1))
                            nc.scalar.copy(
                                out=kt[:, ec * T + th * TH: ec * T + (th + 1) * TH],
                                in_=psp[:, 0:TH])

                    # V [t(8x125 part), e]
                    vt = v_p.tile([128, KC * D], BF16, tag="v")
                    for tc2 in range(KC // 2):
                        psp = ps_proj.tile([128, 512], F32, tag="proj")
                        for sub in range(2):
                            tc_ = 2 * tc2 + sub
                            for dc in range(2):
                                nc.tensor.matmul(
                                    psp[0:KCS, sub * D:(sub + 1) * D],
                                    lhsT=ht[:, dc * T + tc_ * KCS: dc * T + (tc_ + 1) * KCS],
                                    rhs=vw_sb[:, i * 2 * D + dc * D: i * 2 * D + (dc + 1) * D],
                                    start=(dc == 0), stop=(dc == 1))
                        nc.vector.tensor_copy(out=vt[0:KCS, tc2 * 2 * D:(tc2 + 1) * 2 * D],
                                              in_=psp[0:KCS, :])

                    # scores^T + exp -> attnT [125 x (kc*512)]
                    att = at_p.tile([128, KC * 512], BF16, tag="at")
                    qbd_b = qbd0 if i == 0 else qtp[p][bb]
                    for kc in range(KC):
                        pss = ps_sc.tile([128, 512], F32, tag="sc")
                        for g in range(2):
                            nc.tensor.matmul(
                                pss[0:KCS, g * 256:(g + 1) * 256],
                                lhsT=kt[:, g * T + kc * KCS: g * T + (kc + 1) * KCS],
                                rhs=qbd_b[:, g * 256:(g + 1) * 256],
                                start=True, stop=True)
                        nc.scalar.activation(out=att[0:KCS, kc * 512:(kc + 1) * 512],
                                             in_=pss[0:KCS, :], func=AF.Exp, scale=SCALE, bias=zero_col[0:KCS, 0:1])

                    # attn@V + softmax sums (cols 256:260)
                    avs = ps_av.tile([128, 512], F32, tag="av")
                    for kc in range(KC):
                        for pp in range(4):
                            lhs_a = att[0:KCS, kc * 512 + pp * 128: kc * 512 + (pp + 1) * 128]
                            nc.tensor.matmul(
                                avs[:, pp * TQ:(pp + 1) * TQ],
                                lhsT=lhs_a,
                                rhs=vt[0:KCS, kc * D + pp * TQ: kc * D + (pp + 1) * TQ],
                                start=(kc == 0 and pp == 0), stop=False)
                            nc.tensor.matmul(
                                avs[:, 256 + pp: 257 + pp],
                                lhsT=lhs_a, rhs=ones_col[0:KCS, 0:1],
                                start=False, stop=(kc == KC - 1 and pp == 3))

                    inv = small_p.tile([128, 4], F32, tag="inv")
                    nc.vector.reciprocal(out=inv[:], in_=avs[:, 256:260])
                    ao = ao_p.tile([64, D], BF16, tag="ao")
                    for pp in range(4):
                        h1, h2 = 2 * pp, 2 * pp + 1
                        nc.vector.tensor_scalar_mul(
                            out=ao[0:64, h1 * 32:(h1 + 1) * 32],
                            in0=avs[0:64, pp * TQ: pp * TQ + 32],
                            scalar1=inv[0:64, pp:pp + 1])
                        nc.vector.tensor_scalar_mul(
                            out=ao[0:64, h2 * 32:(h2 + 1) * 32],
                            in0=avs[64:128, pp * TQ + 32: (pp + 1) * TQ],
                            scalar1=inv[64:128, pp:pp + 1])

                    # transpose attn_out -> aoT pair tile [128, (c*128 + bb*64)]
                    for c in range(2):
                        tp = ps_qs.tile([128, 128], BF16, tag="qs")
                        nc.tensor.transpose(tp[:, 0:TQ], ao[0:TQ, c * 128:(c + 1) * 128],
                                            id_sb[0:TQ, 0:TQ])
                        nc.vector.tensor_copy(
                            out=aoT[:, c * 128 + bb * TQ: c * 128 + (bb + 1) * TQ],
                            in_=tp[:, 0:TQ])

                # ---- o-proj + residual + LN (per pair) ----
                pso = ps_qs.tile([128, 512], F32, tag="qs")
                nc.tensor.matmul(pso[:, 0:D], lhsT=ones_row[0:1, 0:128],
                                 rhs=rows_sb[0:1, RO_OBP + i * D: RO_OBP + (i + 1) * D],
                                 start=True, stop=False)
                for ec in range(2):
                    nc.tensor.matmul(pso[:, 0:D],
                                     lhsT=aoT[:, ec * 128:(ec + 1) * 128],
                                     rhs=ow_sb[:, i * 2 * D + ec * D: i * 2 * D + (ec + 1) * D],
                                     start=False, stop=(ec == 1))
                q_prev = tqpair_sb if i == 0 else qstate[p]
                r_sb = tmp_p.tile([128, D], F32, tag="r")
                nc.vector.tensor_add(out=r_sb[:], in0=pso[:, 0:D], in1=q_prev[:])
                st = small_p.tile([128, 6], F32, tag="st")
                nc.vector.bn_stats(out=st[:], in_=r_sb[:])
                mv = small_p.tile([128, 2], F32, tag="mv")
                nc.vector.bn_aggr(out=mv[:], in_=st[:])
                rstd = small_p.tile([128, 2], F32, tag="rstd")
                nc.scalar.activation(out=rstd[:, 0:1], in_=mv[:, 1:2], func=AF.Ln,
                                     bias=eps_col[:, 0:1], scale=1.0)
                nc.scalar.activation(out=rstd[:, 1:2], in_=rstd[:, 0:1], func=AF.Exp,
                                     bias=zero_col[:, 0:1], scale=-0.5)
                lo = ln_p.tile([128, D], F32, tag="ln")
                nc.vector.tensor_scalar(out=lo[:], in0=r_sb[:],
                                        scalar1=mv[:, 0:1], scalar2=rstd[:, 1:2],
                                        op0=mybir.AluOpType.subtract,
                                        op1=mybir.AluOpType.mult)
                nc.vector.tensor_mul(out=lo[:], in0=lo[:], in1=lns_sb[:, i * D:(i + 1) * D])
                nc.vector.tensor_add(out=lo[:], in0=lo[:], in1=lnb_sb[:, i * D:(i + 1) * D])
                lnout[p] = lo

            # ======== ffn phase ========
            for p in g_pairs:
                lo = lnout[p]
                lcast = tmp_p.tile([128, D], BF16, tag="lcast")
                nc.vector.tensor_copy(out=lcast[:], in_=lo[:])
                lnT = tmp_p.tile([128, D], BF16, tag="lnT")
                pe_transpose_2(lcast, lnT, 0, 128)
                gel = gel_p.tile([128, 8 * 128], BF16, tag="gel")
                for half in range(2):
                    psf = ps_qs.tile([128, 512], F32, tag="qs")
                    for fl in range(4):
                        fc = half * 4 + fl
                        nc.tensor.matmul(
                            psf[:, fl * 128:(fl + 1) * 128],
                            lhsT=rows_sb[0:1, RO_F1B + i * D_FF + fc * 128: RO_F1B + i * D_FF + (fc + 1) * 128],
                            rhs=ones_row[0:1, 0:128], start=True, stop=False)
                        for dc in range(2):
                            nc.tensor.matmul(
                                psf[:, fl * 128:(fl + 1) * 128],
                                lhsT=f1w_sb[:, i * 2 * D_FF + dc * D_FF + fc * 128: i * 2 * D_FF + dc * D_FF + (fc + 1) * 128],
                                rhs=lnT[:, dc * 128:(dc + 1) * 128],
                                start=False, stop=(dc == 1))
                    nc.scalar.activation(out=gel[:, half * 512:(half + 1) * 512],
                                         in_=psf[:], func=GELU[0], bias=zero_col[:, 0:1])
                ps2 = ps_qs.tile([128, 512], F32, tag="qs")
                nc.tensor.matmul(ps2[:, 0:D], lhsT=ones_row[0:1, 0:128],
                                 rhs=rows_sb[0:1, RO_F2B + i * D: RO_F2B + (i + 1) * D],
                                 start=True, stop=False)
                for fc in range(8):
                    nc.tensor.matmul(ps2[:, 0:D],
                                     lhsT=gel[:, fc * 128:(fc + 1) * 128],
                                     rhs=f2w_sb[:, i * 8 * D + fc * D: i * 8 * D + (fc + 1) * D],
                                     start=False, stop=(fc == 7))
                qn = qs_p.tile([128, D], F32, tag="qn")
                nc.vector.tensor_add(out=qn[:], in0=ps2[:, 0:D], in1=lo[:])
                qstate[p] = qn

        # ======== head phase (per group) ========
        for p in g_pairs:
            qcast = tmp_p.tile([128, D], BF16, tag="hcast")
            nc.vector.tensor_copy(out=qcast[:], in_=qstate[p][:])
            qfT = tmp_p.tile([128, D], BF16, tag="qfT")
            pe_transpose_2(qcast, qfT, 0, 128)
            psh = ps_qs.tile([128, 512], F32, tag="qs")
            nc.tensor.matmul(psh[:], lhsT=ones_row[0:1, 0:128],
                             rhs=rows_sb[0:1, RO_BOUT: RO_BOUT + D_OUT],
                             start=True, stop=False)
            for dc in range(2):
                nc.tensor.matmul(psh[:],
                                 lhsT=qfT[:, dc * 128:(dc + 1) * 128],
                                 rhs=wout_sb[:, dc * D_OUT:(dc + 1) * D_OUT],
                                 start=False, stop=(dc == 1))
            osb = outp_p.tile([128, D_OUT], F32, tag="osb")
            nc.vector.tensor_copy(out=osb[:], in_=psh[:])
            nc.sync.dma_start(out=out_d[2 * p: 2 * p + 2, :, :], in_=osb[:])


_CACHE = {}


def _build():
    if "nc" in _CACHE:
        return _CACHE["nc"]
    nc = bacc.Bacc("TRN2", target_bir_lowering=False, debug=False,
                   num_devices=N_CORES)
    ins = {}

    def din(name, shape):
        ins[name] = nc.dram_tensor(name, list(shape), F32, kind="ExternalInput").ap()

    din("xt", (B_LOC, D_IN, T))
    din("post", (D, T))
    din("tqt", (D, TQ))
    din("tqpair", (128, D))
    din("lns", (L, 128, D))
    din("lnb", (L, 128, D))
    din("binv", (D,))
    din("vbv", (L, D))
    din("win", (D_IN, D))
    din("qw", (L, D, D))
    din("kw", (L, D, D))
    din("vw", (L, D, D))
    din("ow", (L, D, D))
    din("qb", (L, D))
    din("ob", (L, D))
    din("f1w", (L, D, D_FF))
    din("f1b", (L, D_FF))
    din("f2w", (L, D_FF, D))
    din("f2b", (L, D))
    din("wout", (D, D_OUT))
    din("bout", (D_OUT,))
    outs = {"out": nc.dram_tensor("out", [B_LOC, TQ, D_OUT], F32,
                                  kind="ExternalOutput").ap()}
    with tile.TileContext(nc) as tc, ExitStack() as ctx:
        _emit(ctx, tc, outs, ins)
    nc.compile()
    _CACHE["nc"] = nc
    return nc


def make_in_maps(inputs):
    """Host-side shard/relayout (pure data movement, no arithmetic)."""
    f = lambda a: np.ascontiguousarray(np.asarray(a), dtype=np.float32)
    x = f(inputs["x"])
    tq = f(inputs["time_queries"])
    pos = f(inputs["pos_encoding"])[:T]
    xt = np.ascontiguousarray(x.transpose(0, 2, 1))          # [B, 512, 1000]
    base = {
        "post": np.ascontiguousarray(pos.T),
        "tqt": np.ascontiguousarray(tq.T),
        "tqpair": np.ascontiguousarray(np.concatenate([tq, tq], axis=0)),
        "lns": np.ascontiguousarray(np.broadcast_to(f(inputs["ln_s"])[:, None, :], (L, 128, D))),
        "lnb": np.ascontiguousarray(np.broadcast_to(f(inputs["ln_b"])[:, None, :], (L, 128, D))),
        "binv": f(inputs["bin_"]),
        "vbv": f(inputs["vb"]),
        "win": f(inputs["win"]),
        "qw": f(inputs["qw"]), "kw": f(inputs["kw"]), "vw": f(inputs["vw"]),
        "ow": f(inputs["ow"]), "qb": f(inputs["qb"]), "ob": f(inputs["ob"]),
        "f1w": f(inputs["f1w"]), "f1b": f(inputs["f1b"]),
        "f2w": f(inputs["f2w"]), "f2b": f(inputs["f2b"]),
        "wout": f(inputs["wout"]), "bout": f(inputs["bout"]),
    }
    in_maps = []
    for c in range(N_CORES):
        m = dict(base)
        m["xt"] = np.ascontiguousarray(xt[c * B_LOC:(c + 1) * B_LOC])
        in_maps.append(m)
    return in_maps


def kernel(**inputs):
    nc = _build()
    in_maps = make_in_maps(inputs)
    res = bass_utils.run_bass_kernel_spmd(nc, in_maps, core_ids=list(range(N_CORES)))
    out = np.concatenate([r["out"] for r in res.results], axis=0)
    return out.astype(np.float32)

